# revision 1
# baseline (speedup 1.0000x reference)
"""2-layer GCN (GCNConv + LayerNorm + ReLU + GCNConv + LayerNorm) on 8 TRN2 NeuronCores.

Strategy:
  - Nodes are degree-sorted and dealt round-robin to 8 cores (uniform degree profiles
    -> identical SPMD schedules). Each core owns 6250 dst nodes (padded to 6272).
  - Per layer: scale local rows by dinv, cast bf16, AllGather -> full 50176-row table
    in each core's DRAM.
  - Aggregation: edges grouped by (dst tile, round) with lane == dst position, so a
    gathered 128-edge chunk accumulates into PSUM via a matmul with a *constant
    identity* stationary: psum[d, f] += G[d, f]. Source rows fetched by gpsimd
    dma_gather (int16 indices -> table split into two 25088-row halves; dst nodes
    re-tiled per half by half-degree to keep padding low). The half-1 partial sums
    are folded into half-0 (storage) tile order with host-built permutation matmuls.
  - Self-loops are added locally (identity matmul of the resident scaled rows).
  - Dense W matmul per tile (transpose via PE), then LayerNorm on f32.
"""
import os
import numpy as np
import ml_dtypes

N = 50000
E = 600000
D = 128
NC = 8
P = 128
SHARD = 6272            # 49 * 128
TILES = 49
HALF_ROWS = SHARD * 4   # 25088 rows per gather half (< int16 range)
LN_EPS = 1e-5
GBUF_CHUNKS = 64        # chunks (128 edges each) per dma_gather call group

bf16 = ml_dtypes.bfloat16


# ----------------------------------------------------------------------------
# Host-side planning (index-only preprocessing)
# ----------------------------------------------------------------------------

class Plan:
    pass


def build_plan(edge_index: np.ndarray) -> Plan:
    pl = Plan()
    src = edge_index[0].astype(np.int64)
    dst = edge_index[1].astype(np.int64)

    deg = np.bincount(dst, minlength=N) + 1          # incl. mandatory self-loop
    order = np.argsort(-deg, kind="stable")          # global degree desc
    core_of = np.empty(N, dtype=np.int64)
    core_of[order] = np.arange(N) % NC               # deal round-robin

    src_half = (core_of[src] >= 4).astype(np.int64)  # 0: table rows 0..25087
    degH = np.zeros((2, N), dtype=np.int64)
    degH[0] = np.bincount(dst[src_half == 0], minlength=N)
    degH[1] = np.bincount(dst[src_half == 1], minlength=N)

    # Storage order per core = H0 order (sorted by degH0 desc); H1 order separate.
    posH = np.empty((2, N), dtype=np.int64)
    node_at = np.full((NC, SHARD), -1, dtype=np.int64)   # storage order
    for c in range(NC):
        shard = order[c::NC]                              # 6250 nodes
        for h in range(2):
            so = np.argsort(-degH[h][shard], kind="stable")
            posH[h][shard[so]] = np.arange(len(shard))
        node_at[c, :len(shard)] = shard[np.argsort(posH[0][shard])]
    pl.node_at = node_at
    pl.deg = deg

    lane_of = posH % P
    tile_of = posH // P

    # per (half, tile): R = max lane count, uniform over cores
    R_uni = np.zeros((2, TILES), dtype=np.int64)
    for h in range(2):
        key = core_of * TILES + tile_of[h]
        m = np.zeros(NC * TILES, dtype=np.int64)
        np.maximum.at(m, key, degH[h])
        R_uni[h] = m.reshape(NC, TILES).max(axis=0)
    pl.R_uni = R_uni

    chunk_base = np.zeros((2, TILES + 1), dtype=np.int64)
    for h in range(2):
        chunk_base[h, 1:] = np.cumsum(R_uni[h])
    pl.chunk_base = chunk_base
    pl.n_chunks = chunk_base[:, -1]

    # round index for each edge: rank among edges with same (half, dst)
    ekey = src_half * N + dst
    eorder = np.argsort(ekey, kind="stable")
    sk = ekey[eorder]
    starts = np.r_[0, np.flatnonzero(sk[1:] != sk[:-1]) + 1]
    group_of = np.zeros(E, dtype=np.int64)
    group_of[starts[1:]] = 1
    group_of = np.cumsum(group_of)
    rounds_sorted = np.arange(E) - starts[group_of]
    rounds = np.empty(E, dtype=np.int64)
    rounds[eorder] = rounds_sorted

    # slot arrays per (core, half): [n_chunks*128] of table indices (pad -> zero row)
    PAD_IDX = 6250                                    # dummy (zero) row
    slots = [[np.full(pl.n_chunks[h] * P, PAD_IDX, dtype=np.int64) for h in range(2)]
             for _ in range(NC)]
    e_core = core_of[dst]
    e_tile = tile_of[src_half, dst]
    e_lane = lane_of[src_half, dst]
    e_slot = (chunk_base[src_half, e_tile] + rounds) * P + e_lane
    e_val = (core_of[src] % 4) * SHARD + posH[0][src]
    for c in range(NC):
        for h in range(2):
            m = (e_core == c) & (src_half == h)
            slots[c][h][e_slot[m]] = e_val[m]

    # call groups per half: greedy fill up to GBUF_CHUNKS
    groups = [[], []]
    for h in range(2):
        cur, cur_n = [], 0
        for t in range(TILES):
            r = int(R_uni[h][t])
            if r == 0:
                continue
            while r > GBUF_CHUNKS:
                if cur:
                    groups[h].append(cur)
                    cur, cur_n = [], 0
                groups[h].append([(t, GBUF_CHUNKS, True)])
                r -= GBUF_CHUNKS
            if cur_n + r > GBUF_CHUNKS and cur:
                groups[h].append(cur)
                cur, cur_n = [], 0
            cur.append((t, r, False))
            cur_n += r
        if cur:
            groups[h].append(cur)
    pl.groups = groups
    sched = [(1, gi) for gi in range(len(groups[1]))]
    sched += [(0, gi) for gi in reversed(range(len(groups[0])))]
    pl.sched = sched

    def wrap(flat):                                    # [num] -> [128, num//16]
        num = len(flat)
        w = np.zeros((16, num // 16), dtype=np.int16)
        w[np.arange(num) % 16, np.arange(num) // 16] = flat.astype(np.int16)
        return np.tile(w, (8, 1))

    # wrapped int16 index tensors [128, total_cols] per core; groups consume
    # consecutive chunk ranges per half, tracked via per-half cursors.
    idx_in = []
    col_ranges = [[], []]
    for c in range(NC):
        parts = []
        col = 0
        cursor = [0, 0]                                # chunk cursor per half
        for h in range(2):
            for grp in groups[h]:
                nch = sum(r for (_, r, _) in grp)
                i0 = cursor[h] * P
                i1 = (cursor[h] + nch) * P
                cursor[h] += nch
                seg = slots[c][h][i0:i1]
                parts.append(wrap(seg))
                if c == 0:
                    col_ranges[h].append((col, col + len(seg) // 16))
                col += len(seg) // 16
        idx_in.append(np.concatenate(parts, axis=1))
    pl.idx_in = idx_in
    pl.col_ranges = col_ranges

    # permutation blocks: fold H1-tiling partials into H0 (storage) tiling.
    # Uniform schedule: per tile t, the list of source H1 tiles = union over cores.
    need = [set() for _ in range(TILES)]
    percore = [[dict() for _ in range(TILES)] for _ in range(NC)]
    for c in range(NC):
        for pidx in range(SHARD):
            n_ = node_at[c, pidx]
            if n_ < 0:
                continue
            q = posH[1][n_]
            t, l = pidx // P, pidx % P
            t1, l1 = q // P, q % P
            need[t].add(t1)
            percore[c][t].setdefault(t1, []).append((l1, l))
    block_t1 = [sorted(need[t]) for t in range(TILES)]
    NBLK = np.array([len(s) for s in block_t1], dtype=np.int64)
    TB = int(NBLK.sum())
    perm_in = []
    tgt_in = []
    for c in range(NC):
        pm = np.zeros((TB, P, P), dtype=np.float32)
        tg = np.full((P, TB), -1.0, dtype=np.float32)
        bi = 0
        for t in range(TILES):
            for t1 in block_t1[t]:
                for (l1, l) in percore[c][t].get(t1, []):
                    pm[bi, l1, l] = 1.0
                    tg[l1, bi] = float(l)
                bi += 1
        perm_in.append(pm.astype(bf16))
        tgt_in.append(tg)
    pl.NBLK = NBLK
    pl.TB = TB
    pl.perm_in = perm_in
    pl.tgt_in = tgt_in
    pl.block_t1 = block_t1
    return pl


# ----------------------------------------------------------------------------
# Numpy emulation of the device program (for validating the plan quickly)
# ----------------------------------------------------------------------------

def emulate(pl, inputs):
    x = np.asarray(inputs["x"], dtype=np.float32)
    W = [np.asarray(inputs["W1"], np.float32), np.asarray(inputs["W2"], np.float32)]
    b = [np.asarray(inputs["b1"], np.float32), np.asarray(inputs["b2"], np.float32)]
    g = [np.asarray(inputs["g1"], np.float32), np.asarray(inputs["g2"], np.float32)]
    be = [np.asarray(inputs["beta1"], np.float32), np.asarray(inputs["beta2"], np.float32)]

    def tobf(a):
        return a.astype(bf16).astype(np.float32)

    deg_t = np.ones((NC, P, TILES), dtype=np.float32)
    xp = np.zeros((NC, SHARD, D), dtype=np.float32)
    for c in range(NC):
        for pidx in range(SHARD):
            n_ = pl.node_at[c, pidx]
            if n_ >= 0:
                deg_t[c, pidx % P, pidx // P] = pl.deg[n_]
                xp[c, pidx] = x[n_]
    dinv = 1.0 / np.sqrt(deg_t)

    cur = xp
    outs = np.zeros((NC, SHARD, D), dtype=np.float32)
    for layer in range(2):
        store = np.zeros((NC, SHARD, D), dtype=np.float32)
        for c in range(NC):
            for t in range(TILES):
                sl = slice(t * P, (t + 1) * P)
                store[c, sl] = tobf(cur[c, sl] * dinv[c, :, t][:, None])
        if layer == 1:
            for c in range(NC):
                for pidx in range(SHARD):
                    if pl.node_at[c, pidx] < 0:
                        store[c, pidx] = 0.0
        table = store.reshape(NC * SHARD, D)
        half_tab = [table[:HALF_ROWS], table[HALF_ROWS:]]

        for c in range(NC):
            H1sb = np.zeros((TILES, P, D), dtype=np.float32)
            psum_part = {}                              # tile -> running psum (H0 split tiles)
            for h in (1, 0):
                cursor = 0
                for gi, grp in enumerate(pl.groups[h]):
                    c0, c1 = pl.col_ranges[h][gi]
                    wrapped = pl.idx_in[c][:16, c0:c1].astype(np.int64)
                    num = (c1 - c0) * 16
                    seg = np.empty(num, dtype=np.int64)
                    seg[np.arange(num)] = wrapped[np.arange(num) % 16, np.arange(num) // 16]
                    Gt = half_tab[h][seg].reshape(num // P, P, D)
                    off = 0
                    for (t, r, partial) in grp:
                        ps = Gt[off:off + r].sum(axis=0)
                        off += r
                        key = (h, t)
                        if key in psum_part:
                            ps = ps + psum_part.pop(key)
                        if partial:
                            psum_part[key] = ps
                            continue
                        if h == 1:
                            H1sb[t] = tobf(ps)
                        else:
                            outs[c, t * P:(t + 1) * P] = _emu_finish(
                                pl, c, t, ps, store, H1sb, dinv, W, b, g, be, layer)
            covered = set(t for grp in pl.groups[0] for (t, _, pa) in grp if not pa)
            for t in range(TILES):
                if t not in covered:
                    ps = np.zeros((P, D), dtype=np.float32)
                    outs[c, t * P:(t + 1) * P] = _emu_finish(
                        pl, c, t, ps, store, H1sb, dinv, W, b, g, be, layer)
        if layer == 0:
            cur = outs.copy()
            outs = np.zeros_like(outs)

    full = np.zeros((N, D), dtype=np.float32)
    for c in range(NC):
        for pidx in range(SHARD):
            n_ = pl.node_at[c, pidx]
            if n_ >= 0:
                full[n_] = outs[c, pidx]
    return full


def _emu_finish(pl, c, t, ps, store, H1sb, dinv, W, b, g, be, layer):
    def tobf(a):
        return a.astype(bf16).astype(np.float32)
    acc = ps + store[c, t * P:(t + 1) * P]
    bi0 = int(pl.NBLK[:t].sum())
    for j, t1 in enumerate(pl.block_t1[t]):
        Pm = np.asarray(pl.perm_in[c][bi0 + j], np.float32)
        acc = acc + Pm.T @ H1sb[t1]
    b_triv = not np.any(b[layer])
    if b_triv:
        s_agg = tobf(acc)
    else:
        s_agg = tobf(acc)
    conv = s_agg @ tobf(W[layer])
    if not b_triv:
        conv = conv * dinv[c, :, t][:, None]
    cb = conv + b[layer][None, :]
    mu = cb.mean(axis=1, keepdims=True)
    ctr = cb - mu
    var = (ctr ** 2).mean(axis=1, keepdims=True)
    o = ctr / np.sqrt(var + LN_EPS) * g[layer][None, :] + be[layer][None, :]
    if layer == 0:
        o = np.maximum(o, 0.0)
    return o


# ----------------------------------------------------------------------------
# Bass kernel
# ----------------------------------------------------------------------------

def build_bass(pl, triv):
    import concourse.bacc as bacc
    import concourse.mybir as mybir
    import concourse.tile as tile
    from concourse.masks import make_identity

    f32 = mybir.dt.float32
    bf = mybir.dt.bfloat16
    AF = mybir.ActivationFunctionType
    OP = mybir.AluOpType

    nc = bacc.Bacc("TRN2", target_bir_lowering=False, debug=False, num_devices=NC)

    x_ext = nc.declare_dram_parameter("x", [SHARD, D], f32, isOutput=False)
    deg_ext = nc.declare_dram_parameter("deg", [P, TILES], f32, isOutput=False)
    totcols = pl.idx_in[0].shape[1]
    idx_ext = nc.declare_dram_parameter("idx", [P, totcols], mybir.dt.int16, isOutput=False)
    tgt_ext = nc.declare_dram_parameter("tgt", [P, pl.TB], f32, isOutput=False)
    W_ext = [nc.declare_dram_parameter(f"W{i+1}", [D, D], f32, isOutput=False) for i in range(2)]
    vecs_ext = {}
    for nm in ["b1", "g1", "beta1", "b2", "g2", "beta2"]:
        vecs_ext[nm] = nc.declare_dram_parameter(nm, [P, D], f32, isOutput=False)
    out_ext = nc.declare_dram_parameter("out", [SHARD, D], f32, isOutput=True)

    with tile.TileContext(nc) as tc:
        with tc.tile_pool(name="const", bufs=1) as cpool, \
             tc.tile_pool(name="store", bufs=1) as spool, \
             tc.tile_pool(name="g", bufs=4) as gpool, \
             tc.tile_pool(name="work", bufs=3) as wpool, \
             tc.tile_pool(name="permp", bufs=4) as ppool, \
             tc.tile_pool(name="psA", bufs=3, space="PSUM") as psA, \
             tc.tile_pool(name="psB", bufs=2, space="PSUM") as psB, \
             tc.tile_pool(name="psC", bufs=2, space="PSUM") as psC, \
             tc.tile_pool(name="dram", bufs=1, space="DRAM") as dpool:

            ident32 = cpool.tile([P, P], f32)
            make_identity(nc, ident32[:])
            ident_bf = cpool.tile([P, P], bf)
            nc.vector.tensor_copy(out=ident_bf[:], in_=ident32[:])

            Wbf = []
            for i in range(2):
                wt = cpool.tile([P, D], f32, name=f"w32_{i}")
                nc.sync.dma_start(out=wt[:], in_=W_ext[i][:])
                wb = cpool.tile([P, D], bf, name=f"wbf_{i}")
                nc.vector.tensor_copy(out=wb[:], in_=wt[:])
                Wbf.append(wb)

            vecs = {}
            for nm in vecs_ext:
                vt = cpool.tile([P, D], f32, name=f"vec_{nm}")
                nc.sync.dma_start(out=vt[:], in_=vecs_ext[nm][:])
                vecs[nm] = vt

            deg_t = cpool.tile([P, TILES], f32)
            nc.sync.dma_start(out=deg_t[:], in_=deg_ext[:])
            sq = cpool.tile([P, TILES], f32)
            nc.scalar.activation(out=sq[:], in_=deg_t[:], func=AF.Sqrt)
            dinv = cpool.tile([P, TILES], f32)
            nc.vector.reciprocal(dinv[:], sq[:])

            eps_t = cpool.tile([P, 1], f32)
            nc.vector.memset(eps_t[:], float(LN_EPS))
            # per-partition mask: 1.0 for real lanes of the last tile, 0.0 for dummies
            ndum = SHARD - 6250
            dmask = cpool.tile([P, 1], f32)
            nc.vector.memset(dmask[:], 1.0)
            nc.gpsimd.affine_select(
                out=dmask[:], in_=dmask[:], pattern=[[0, 1]],
                compare_op=OP.is_ge, fill=0.0,
                base=P - ndum - 1, channel_multiplier=-1)
            idx_t = cpool.tile([P, totcols], mybir.dt.int16)
            nc.sync.dma_start(out=idx_t[:], in_=idx_ext[:])
            tgt_t = cpool.tile([P, pl.TB], f32)
            nc.sync.dma_start(out=tgt_t[:], in_=tgt_ext[:])
            iota_i = cpool.tile([P, P], mybir.dt.int32)
            nc.gpsimd.iota(iota_i[:], pattern=[[1, P]], base=0, channel_multiplier=0)
            iota_row = cpool.tile([P, P], f32)
            nc.vector.tensor_copy(out=iota_row[:], in_=iota_i[:])

            x_store = spool.tile([P, TILES, D], f32)
            nc.sync.dma_start(out=x_store[:],
                              in_=x_ext[:].rearrange("(t l) f -> l t f", t=TILES))
            xs_store = spool.tile([P, TILES, D], bf)
            for t in range(TILES):
                nc.scalar.activation(out=xs_store[:, t, :], in_=x_store[:, t, :],
                                     func=AF.Identity, scale=dinv[:, t:t + 1])

            dinvm = cpool.tile([P, 1], f32)
            nc.vector.tensor_scalar(out=dinvm[:], in0=dinv[:, TILES - 1:TILES],
                                    scalar1=dmask[:, 0:1], scalar2=None, op0=OP.mult)
            h1g_store = spool.tile([P, TILES, D], bf)
            H1sb = spool.tile([P, TILES, D], bf)

            cc_in = [dpool.tile([SHARD, D], bf, name=f"ccin{i}") for i in range(2)]
            cc_out = [dpool.tile([NC * SHARD, D], bf, name=f"ccout{i}",
                                 addr_space="Shared") for i in range(2)]

            NBMAX = int(pl.NBLK.max())

            def finish_h0_tile(layer, t, ps, started):
                selfstore = xs_store if layer == 0 else h1g_store
                nc.tensor.matmul(out=ps[:], lhsT=ident_bf[:],
                                 rhs=selfstore[:, t, :], start=not started, stop=False)
                bi0 = int(pl.NBLK[:t].sum())
                nb = len(pl.block_t1[t])
                assert nb >= 1
                pb = ppool.tile([P, NBMAX, P], bf, tag="pb", name=f"pb_{layer}_{t}")
                for j in range(nb):
                    nc.vector.tensor_scalar(out=pb[:, j, :], in0=iota_row[:],
                                            scalar1=tgt_t[:, bi0 + j:bi0 + j + 1],
                                            scalar2=None, op0=OP.is_equal)
                for j, t1 in enumerate(pl.block_t1[t]):
                    nc.tensor.matmul(out=ps[:], lhsT=pb[:, j, :], rhs=H1sb[:, t1, :],
                                     start=False, stop=(j == nb - 1))

                # evict agg -> bf16 (no dinv scale: LN is scale-invariant when b==0)
                b_triv, g_triv, be_triv = triv[layer]
                s_agg = wpool.tile([P, D], bf, tag="sagg", name=f"sagg_{layer}_{t}")
                if b_triv:
                    nc.scalar.activation(out=s_agg[:], in_=ps[:], func=AF.Identity)
                else:
                    nc.scalar.activation(out=s_agg[:], in_=ps[:], func=AF.Identity,
                                         scale=dinv[:, t:t + 1])
                psT = psB.tile([P, D], bf, space="PSUM", tag="tr",
                               name=f"psT_{layer}_{t}")
                nc.tensor.transpose(out=psT[:], in_=s_agg[:], identity=ident_bf[:])
                s_aggT = wpool.tile([P, D], bf, tag="saggT", name=f"saggT_{layer}_{t}")
                nc.vector.tensor_copy(out=s_aggT[:], in_=psT[:])
                convp = psC.tile([P, D], f32, space="PSUM", tag="conv",
                                 name=f"conv_{layer}_{t}")
                nc.tensor.matmul(out=convp[:], lhsT=s_aggT[:], rhs=Wbf[layer][:],
                                 start=True, stop=True)

                if b_triv:
                    cb_ap = convp[:]
                else:
                    bv = vecs["b1" if layer == 0 else "b2"]
                    cb = wpool.tile([P, D], f32, tag="cb", name=f"cb_{layer}_{t}")
                    nc.vector.tensor_tensor(out=cb[:], in0=convp[:], in1=bv[:], op=OP.add)
                    cb_ap = cb[:]
                scr = wpool.tile([P, D], f32, tag="scr", name=f"scr_{layer}_{t}")
                negmu = wpool.tile([P, 1], f32, tag="negmu", name=f"negmu_{layer}_{t}")
                nc.scalar.activation(out=scr[:], in_=cb_ap, func=AF.Identity,
                                     scale=-1.0 / D, accum_out=negmu[:])
                ctr = wpool.tile([P, D], f32, tag="ctr", name=f"ctr_{layer}_{t}")
                nc.scalar.activation(out=ctr[:], in_=cb_ap, func=AF.Identity,
                                     bias=negmu[:, 0:1])
                sqs = wpool.tile([P, D], f32, tag="sqs", name=f"sqs_{layer}_{t}")
                var_raw = wpool.tile([P, 1], f32, tag="varr", name=f"varr_{layer}_{t}")
                nc.scalar.activation(out=sqs[:], in_=ctr[:], func=AF.Square,
                                     scale=float(1.0 / np.sqrt(D)),
                                     accum_out=var_raw[:])
                std = wpool.tile([P, 1], f32, tag="std", name=f"std_{layer}_{t}")
                nc.scalar.activation(out=std[:], in_=var_raw[:], func=AF.Sqrt,
                                     bias=eps_t[:, 0:1])
                rstd = wpool.tile([P, 1], f32, tag="rstd", name=f"rstd_{layer}_{t}")
                nc.vector.reciprocal(rstd[:], std[:])

                if not (g_triv and be_triv):
                    gv = vecs["g1" if layer == 0 else "g2"]
                    bev = vecs["beta1" if layer == 0 else "beta2"]
                    o1 = wpool.tile([P, D], f32, tag="o1", name=f"o1_{layer}_{t}")
                    nc.scalar.activation(out=o1[:], in_=ctr[:], func=AF.Identity,
                                         scale=rstd[:, 0:1])
                    o2 = wpool.tile([P, D], f32, tag="o2", name=f"o2_{layer}_{t}")
                    nc.vector.tensor_tensor(out=o2[:], in0=o1[:], in1=gv[:], op=OP.mult)
                    o3 = wpool.tile([P, D], f32, tag="o3", name=f"o3_{layer}_{t}")
                    nc.vector.tensor_tensor(out=o3[:], in0=o2[:], in1=bev[:], op=OP.add)
                    if layer == 0:
                        o4 = wpool.tile([P, D], f32, tag="o4", name=f"o4_{t}")
                        nc.scalar.activation(out=o4[:], in_=o3[:], func=AF.Relu)
                        dcol = dinvm[:, 0:1] if t == TILES - 1 else dinv[:, t:t + 1]
                        nc.vector.tensor_scalar(out=h1g_store[:, t, :], in0=o4[:],
                                                scalar1=dcol, scalar2=None,
                                                op0=OP.mult)
                    else:
                        nc.sync.dma_start(out=out_ext[t * P:(t + 1) * P, :], in_=o3[:])
                else:
                    if layer == 0:
                        dcol = dinvm[:, 0:1] if t == TILES - 1 else dinv[:, t:t + 1]
                        rsd = wpool.tile([P, 1], f32, tag="rsd", name=f"rsd_{t}")
                        nc.vector.tensor_scalar(out=rsd[:], in0=rstd[:],
                                                scalar1=dcol, scalar2=None,
                                                op0=OP.mult)
                        nc.scalar.activation(out=h1g_store[:, t, :], in_=ctr[:],
                                             func=AF.Relu, scale=rsd[:, 0:1])
                    else:
                        o1 = wpool.tile([P, D], f32, tag="o1", name=f"o1_{layer}_{t}")
                        nc.scalar.activation(out=o1[:], in_=ctr[:], func=AF.Identity,
                                             scale=rstd[:, 0:1])
                        nc.sync.dma_start(out=out_ext[t * P:(t + 1) * P, :], in_=o1[:])

            def run_layer(layer):
                selfstore = xs_store if layer == 0 else h1g_store
                for t0 in range(0, TILES, 7):
                    t1b = min(t0 + 7, TILES)
                    nc.sync.dma_start(
                        out=cc_in[layer][t0 * P:t1b * P, :].rearrange(
                            "(t l) f -> l t f", t=t1b - t0),
                        in_=selfstore[:, t0:t1b, :])
                nc.gpsimd.collective_compute(
                    "AllGather", OP.bypass,
                    replica_groups=[list(range(NC))],
                    ins=[cc_in[layer][:].opt()],
                    outs=[cc_out[layer][:].opt()],
                )
                table = cc_out[layer]
                nc.vector.memset(H1sb[:], 0.0)

                open_ps = {}                 # (h, t) -> (ps tile, started)
                if True:
                    for (h, gi) in pl.sched:
                        half_ap = table[HALF_ROWS:, :] if h == 1 else table[:HALF_ROWS, :]
                        grp = pl.groups[h][gi]
                        c0, c1 = pl.col_ranges[h][gi]
                        nch = sum(r for (_, r, _) in grp)
                        gbuf = gpool.tile([P, GBUF_CHUNKS, D], bf, tag="g",
                                          name=f"g_{layer}_{h}_{gi}")
                        nc.gpsimd.dma_gather(
                            out_ap=gbuf[:, :nch, :],
                            in_ap=half_ap,
                            idxs_ap=idx_t[:, c0:c1],
                            num_idxs=nch * P,
                            num_idxs_reg=nch * P,
                            elem_size=D,
                            single_packet=False,
                        )
                        off = 0
                        for (t, r, partial) in grp:
                            key = (h, t)
                            if key in open_ps:
                                ps, started = open_ps.pop(key)
                            else:
                                ps = psA.tile([P, D], f32, space="PSUM", tag="agg",
                                              name=f"ps_{layer}_{h}_{t}")
                                started = False
                            for ri in range(r):
                                last = (not partial) and (h == 1) and (ri == r - 1)
                                nc.tensor.matmul(out=ps[:], lhsT=ident_bf[:],
                                                 rhs=gbuf[:, off + ri, :],
                                                 start=not started, stop=last)
                                started = True
                            off += r
                            if partial:
                                open_ps[key] = (ps, started)
                            elif h == 1:
                                nc.scalar.activation(out=H1sb[:, t, :], in_=ps[:],
                                                     func=AF.Identity)
                            else:
                                finish_h0_tile(layer, t, ps, started)
                covered = set(t for grp in pl.groups[0] for (t, _, pa) in grp if not pa)
                for t in range(TILES):
                    if t not in covered:
                        ps = psA.tile([P, D], f32, space="PSUM", tag="agg",
                                      name=f"ps_{layer}_0z_{t}")
                        finish_h0_tile(layer, t, ps, False)

            run_layer(0)
            run_layer(1)

    nc.compile()
    return nc


# ----------------------------------------------------------------------------
# Entry point
# ----------------------------------------------------------------------------

_last_result = None


def kernel(**inputs) -> np.ndarray:
    edge_index = np.asarray(inputs["edge_index"])
    pl = build_plan(edge_index)

    if os.environ.get("KERNEL_EMULATE") == "1":
        return emulate(pl, inputs)

    from concourse.bass_utils import run_bass_kernel_spmd
    triv = []
    for i in (1, 2):
        triv.append((
            not np.any(np.asarray(inputs[f"b{i}"])),
            np.all(np.asarray(inputs[f"g{i}"]) == 1.0),
            not np.any(np.asarray(inputs[f"beta{i}"])),
        ))
    nc = build_bass(pl, triv)

    x = np.asarray(inputs["x"], dtype=np.float32)
    in_maps = []
    for c in range(NC):
        deg_t = np.ones((P, TILES), dtype=np.float32)
        xp = np.zeros((SHARD, D), dtype=np.float32)
        valid = pl.node_at[c] >= 0
        pidx = np.arange(SHARD)
        deg_t[pidx[valid] % P, pidx[valid] // P] = pl.deg[pl.node_at[c][valid]]
        xp[valid] = x[pl.node_at[c][valid]]
        m = {
            "x": xp,
            "deg": deg_t,
            "idx": pl.idx_in[c],
            "tgt": pl.tgt_in[c],
            "W1": np.asarray(inputs["W1"], np.float32),
            "W2": np.asarray(inputs["W2"], np.float32),
        }
        for nm in ["b1", "g1", "beta1", "b2", "g2", "beta2"]:
            m[nm] = np.tile(np.asarray(inputs[nm], np.float32)[None, :], (P, 1))
        in_maps.append(m)

    kw = {}
    if os.environ.get("KERNEL_TRACE") == "1":
        kw = dict(trace=True, trace_cores=[0])
    res = run_bass_kernel_spmd(nc, in_maps, core_ids=list(range(NC)), **kw)
    global _last_result
    _last_result = res

    out = np.zeros((N, D), dtype=np.float32)
    for c in range(NC):
        o = np.asarray(res.results[c]["out"], dtype=np.float32)
        valid = pl.node_at[c] >= 0
        out[pl.node_at[c][valid]] = o[valid]
    return out



# revision 4
# speedup vs baseline: 1.4375x; 1.4375x over previous
"""2-layer GCN (GCNConv + LayerNorm + ReLU + GCNConv + LayerNorm) on 8 TRN2 NeuronCores.

Strategy:
  - Nodes are degree-sorted and dealt round-robin to 8 cores (uniform degree profiles
    -> identical SPMD schedules). Each core owns 6250 dst nodes (padded to 6272).
  - Per layer: scale local rows by dinv, cast bf16, AllGather -> full 50176-row table
    in each core's DRAM.
  - Aggregation: edges grouped by (dst tile, round) with lane == dst position, so a
    gathered 128-edge chunk accumulates into PSUM via a matmul with a *constant
    identity* stationary: psum[d, f] += G[d, f]. Source rows fetched by gpsimd
    dma_gather (int16 indices -> table split into two 25088-row halves; dst nodes
    re-tiled per half by half-degree to keep padding low). The half-1 partial sums
    are folded into half-0 (storage) tile order with host-built permutation matmuls.
  - Self-loops are added locally (identity matmul of the resident scaled rows).
  - Dense W matmul per tile (transpose via PE), then LayerNorm on f32.
"""
import os
import numpy as np
import ml_dtypes

N = 50000
E = 600000
D = 128
NC = 8
P = 128
SHARD = 6272            # 49 * 128
TILES = 49
HALF_ROWS = SHARD * 4   # 25088 rows per gather half (< int16 range)
LN_EPS = 1e-5
GBUF_CHUNKS = 64        # chunks (128 edges each) per dma_gather call group

bf16 = ml_dtypes.bfloat16


# ----------------------------------------------------------------------------
# Host-side planning (index-only preprocessing)
# ----------------------------------------------------------------------------

class Plan:
    pass


def build_plan(edge_index: np.ndarray) -> Plan:
    pl = Plan()
    src = edge_index[0].astype(np.int64)
    dst = edge_index[1].astype(np.int64)

    deg = np.bincount(dst, minlength=N) + 1          # incl. mandatory self-loop
    order = np.argsort(-deg, kind="stable")          # global degree desc
    core_of = np.empty(N, dtype=np.int64)
    core_of[order] = np.arange(N) % NC               # deal round-robin

    src_half = (core_of[src] >= 4).astype(np.int64)  # 0: table rows 0..25087
    degH = np.zeros((2, N), dtype=np.int64)
    degH[0] = np.bincount(dst[src_half == 0], minlength=N)
    degH[1] = np.bincount(dst[src_half == 1], minlength=N)

    # Storage order per core = H0 order (sorted by degH0 desc); H1 order separate.
    posH = np.empty((2, N), dtype=np.int64)
    node_at = np.full((NC, SHARD), -1, dtype=np.int64)   # storage order
    for c in range(NC):
        shard = order[c::NC]                              # 6250 nodes
        for h in range(2):
            so = np.argsort(-degH[h][shard], kind="stable")
            posH[h][shard[so]] = np.arange(len(shard))
        node_at[c, :len(shard)] = shard[np.argsort(posH[0][shard])]
    pl.node_at = node_at
    pl.deg = deg

    lane_of = posH % P
    tile_of = posH // P

    # per (half, tile): R = max lane count, uniform over cores
    R_uni = np.zeros((2, TILES), dtype=np.int64)
    for h in range(2):
        key = core_of * TILES + tile_of[h]
        m = np.zeros(NC * TILES, dtype=np.int64)
        np.maximum.at(m, key, degH[h])
        R_uni[h] = m.reshape(NC, TILES).max(axis=0)
    pl.R_uni = R_uni

    chunk_base = np.zeros((2, TILES + 1), dtype=np.int64)
    for h in range(2):
        chunk_base[h, 1:] = np.cumsum(R_uni[h])
    pl.chunk_base = chunk_base
    pl.n_chunks = chunk_base[:, -1]

    # round index for each edge: rank among edges with same (half, dst)
    ekey = src_half * N + dst
    eorder = np.argsort(ekey, kind="stable")
    sk = ekey[eorder]
    starts = np.r_[0, np.flatnonzero(sk[1:] != sk[:-1]) + 1]
    group_of = np.zeros(E, dtype=np.int64)
    group_of[starts[1:]] = 1
    group_of = np.cumsum(group_of)
    rounds_sorted = np.arange(E) - starts[group_of]
    rounds = np.empty(E, dtype=np.int64)
    rounds[eorder] = rounds_sorted

    # slot arrays per (core, half): [n_chunks*128] of table indices (pad -> zero row)
    PAD_IDX = 6250                                    # dummy (zero) row
    slots = [[np.full(pl.n_chunks[h] * P, PAD_IDX, dtype=np.int64) for h in range(2)]
             for _ in range(NC)]
    e_core = core_of[dst]
    e_tile = tile_of[src_half, dst]
    e_lane = lane_of[src_half, dst]
    e_slot = (chunk_base[src_half, e_tile] + rounds) * P + e_lane
    e_val = (core_of[src] % 4) * SHARD + posH[0][src]
    for c in range(NC):
        for h in range(2):
            m = (e_core == c) & (src_half == h)
            slots[c][h][e_slot[m]] = e_val[m]

    # call groups per half: greedy fill up to GBUF_CHUNKS
    groups = [[], []]
    for h in range(2):
        cur, cur_n = [], 0
        for t in range(TILES):
            r = int(R_uni[h][t])
            if r == 0:
                continue
            while r > GBUF_CHUNKS:
                if cur:
                    groups[h].append(cur)
                    cur, cur_n = [], 0
                groups[h].append([(t, GBUF_CHUNKS, True)])
                r -= GBUF_CHUNKS
            if cur_n + r > GBUF_CHUNKS and cur:
                groups[h].append(cur)
                cur, cur_n = [], 0
            cur.append((t, r, False))
            cur_n += r
        if cur:
            groups[h].append(cur)
    pl.groups = groups
    sched = [(1, gi) for gi in range(len(groups[1]))]
    sched += [(0, gi) for gi in reversed(range(len(groups[0])))]
    pl.sched = sched

    def wrap(flat):                                    # [num] -> [128, num//16]
        num = len(flat)
        w = np.zeros((16, num // 16), dtype=np.int16)
        w[np.arange(num) % 16, np.arange(num) // 16] = flat.astype(np.int16)
        return np.tile(w, (8, 1))

    # wrapped int16 index tensors [128, total_cols] per core; groups consume
    # consecutive chunk ranges per half, tracked via per-half cursors.
    idx_in = []
    col_ranges = [[], []]
    for c in range(NC):
        parts = []
        col = 0
        cursor = [0, 0]                                # chunk cursor per half
        for h in range(2):
            for grp in groups[h]:
                nch = sum(r for (_, r, _) in grp)
                i0 = cursor[h] * P
                i1 = (cursor[h] + nch) * P
                cursor[h] += nch
                seg = slots[c][h][i0:i1]
                parts.append(wrap(seg))
                if c == 0:
                    col_ranges[h].append((col, col + len(seg) // 16))
                col += len(seg) // 16
        idx_in.append(np.concatenate(parts, axis=1))
    pl.idx_in = idx_in
    pl.col_ranges = col_ranges

    # permutation blocks: fold H1-tiling partials into H0 (storage) tiling.
    # Uniform schedule: per tile t, the list of source H1 tiles = union over cores.
    need = [set() for _ in range(TILES)]
    percore = [[dict() for _ in range(TILES)] for _ in range(NC)]
    for c in range(NC):
        for pidx in range(SHARD):
            n_ = node_at[c, pidx]
            if n_ < 0:
                continue
            q = posH[1][n_]
            t, l = pidx // P, pidx % P
            t1, l1 = q // P, q % P
            need[t].add(t1)
            percore[c][t].setdefault(t1, []).append((l1, l))
    block_t1 = [sorted(need[t]) for t in range(TILES)]
    NBLK = np.array([len(s) for s in block_t1], dtype=np.int64)
    TB = int(NBLK.sum())
    perm_in = []
    tgt_in = []
    for c in range(NC):
        pm = np.zeros((TB, P, P), dtype=np.float32)
        tg = np.full((P, TB), -1.0, dtype=np.float32)
        bi = 0
        for t in range(TILES):
            for t1 in block_t1[t]:
                for (l1, l) in percore[c][t].get(t1, []):
                    pm[bi, l1, l] = 1.0
                    tg[l1, bi] = float(l)
                bi += 1
        perm_in.append(pm.astype(bf16))
        tgt_in.append(tg)
    pl.NBLK = NBLK
    pl.TB = TB
    pl.perm_in = perm_in
    pl.tgt_in = tgt_in
    pl.block_t1 = block_t1
    return pl


# ----------------------------------------------------------------------------
# Numpy emulation of the device program (for validating the plan quickly)
# ----------------------------------------------------------------------------

def emulate(pl, inputs):
    x = np.asarray(inputs["x"], dtype=np.float32)
    W = [np.asarray(inputs["W1"], np.float32), np.asarray(inputs["W2"], np.float32)]
    b = [np.asarray(inputs["b1"], np.float32), np.asarray(inputs["b2"], np.float32)]
    g = [np.asarray(inputs["g1"], np.float32), np.asarray(inputs["g2"], np.float32)]
    be = [np.asarray(inputs["beta1"], np.float32), np.asarray(inputs["beta2"], np.float32)]

    def tobf(a):
        return a.astype(bf16).astype(np.float32)

    deg_t = np.ones((NC, P, TILES), dtype=np.float32)
    xp = np.zeros((NC, SHARD, D), dtype=np.float32)
    for c in range(NC):
        for pidx in range(SHARD):
            n_ = pl.node_at[c, pidx]
            if n_ >= 0:
                deg_t[c, pidx % P, pidx // P] = pl.deg[n_]
                xp[c, pidx] = x[n_]
    dinv = 1.0 / np.sqrt(deg_t)

    cur = xp
    outs = np.zeros((NC, SHARD, D), dtype=np.float32)
    for layer in range(2):
        store = np.zeros((NC, SHARD, D), dtype=np.float32)
        for c in range(NC):
            for t in range(TILES):
                sl = slice(t * P, (t + 1) * P)
                store[c, sl] = tobf(cur[c, sl] * dinv[c, :, t][:, None])
        if layer == 1:
            for c in range(NC):
                for pidx in range(SHARD):
                    if pl.node_at[c, pidx] < 0:
                        store[c, pidx] = 0.0
        table = store.reshape(NC * SHARD, D)
        half_tab = [table[:HALF_ROWS], table[HALF_ROWS:]]

        for c in range(NC):
            H1sb = np.zeros((TILES, P, D), dtype=np.float32)
            psum_part = {}                              # tile -> running psum (H0 split tiles)
            for h in (1, 0):
                cursor = 0
                for gi, grp in enumerate(pl.groups[h]):
                    c0, c1 = pl.col_ranges[h][gi]
                    wrapped = pl.idx_in[c][:16, c0:c1].astype(np.int64)
                    num = (c1 - c0) * 16
                    seg = np.empty(num, dtype=np.int64)
                    seg[np.arange(num)] = wrapped[np.arange(num) % 16, np.arange(num) // 16]
                    Gt = half_tab[h][seg].reshape(num // P, P, D)
                    off = 0
                    for (t, r, partial) in grp:
                        ps = Gt[off:off + r].sum(axis=0)
                        off += r
                        key = (h, t)
                        if key in psum_part:
                            ps = ps + psum_part.pop(key)
                        if partial:
                            psum_part[key] = ps
                            continue
                        if h == 1:
                            H1sb[t] = tobf(ps)
                        else:
                            outs[c, t * P:(t + 1) * P] = _emu_finish(
                                pl, c, t, ps, store, H1sb, dinv, W, b, g, be, layer)
            covered = set(t for grp in pl.groups[0] for (t, _, pa) in grp if not pa)
            for t in range(TILES):
                if t not in covered:
                    ps = np.zeros((P, D), dtype=np.float32)
                    outs[c, t * P:(t + 1) * P] = _emu_finish(
                        pl, c, t, ps, store, H1sb, dinv, W, b, g, be, layer)
        if layer == 0:
            cur = outs.copy()
            outs = np.zeros_like(outs)

    full = np.zeros((N, D), dtype=np.float32)
    for c in range(NC):
        for pidx in range(SHARD):
            n_ = pl.node_at[c, pidx]
            if n_ >= 0:
                full[n_] = outs[c, pidx]
    return full


def _emu_finish(pl, c, t, ps, store, H1sb, dinv, W, b, g, be, layer):
    def tobf(a):
        return a.astype(bf16).astype(np.float32)
    acc = ps + store[c, t * P:(t + 1) * P]
    bi0 = int(pl.NBLK[:t].sum())
    for j, t1 in enumerate(pl.block_t1[t]):
        Pm = np.asarray(pl.perm_in[c][bi0 + j], np.float32)
        acc = acc + Pm.T @ H1sb[t1]
    b_triv = not np.any(b[layer])
    if b_triv:
        s_agg = tobf(acc)
    else:
        s_agg = tobf(acc)
    conv = s_agg @ tobf(W[layer])
    if not b_triv:
        conv = conv * dinv[c, :, t][:, None]
    cb = conv + b[layer][None, :]
    mu = cb.mean(axis=1, keepdims=True)
    ctr = cb - mu
    var = (ctr ** 2).mean(axis=1, keepdims=True)
    o = ctr / np.sqrt(var + LN_EPS) * g[layer][None, :] + be[layer][None, :]
    if layer == 0:
        o = np.maximum(o, 0.0)
    return o


# ----------------------------------------------------------------------------
# Bass kernel
# ----------------------------------------------------------------------------

def build_bass(pl, triv):
    import concourse.bacc as bacc
    import concourse.mybir as mybir
    import concourse.tile as tile
    from concourse.masks import make_identity

    f32 = mybir.dt.float32
    bf = mybir.dt.bfloat16
    AF = mybir.ActivationFunctionType
    OP = mybir.AluOpType

    nc = bacc.Bacc("TRN2", target_bir_lowering=False, debug=False, num_devices=NC,
                   num_swdge_queues=4, dynamic_dma_scratch_size=16384)

    x_ext = nc.declare_dram_parameter("x", [SHARD, D], f32, isOutput=False)
    deg_ext = nc.declare_dram_parameter("deg", [P, TILES], f32, isOutput=False)
    totcols = pl.idx_in[0].shape[1]
    idx_ext = nc.declare_dram_parameter("idx", [P, totcols], mybir.dt.int16, isOutput=False)
    tgt_ext = nc.declare_dram_parameter("tgt", [P, pl.TB], f32, isOutput=False)
    W_ext = [nc.declare_dram_parameter(f"W{i+1}", [D, D], f32, isOutput=False) for i in range(2)]
    vecs_ext = {}
    for nm in ["b1", "g1", "beta1", "b2", "g2", "beta2"]:
        vecs_ext[nm] = nc.declare_dram_parameter(nm, [P, D], f32, isOutput=False)
    out_ext = nc.declare_dram_parameter("out", [SHARD, D], f32, isOutput=True)

    with tile.TileContext(nc) as tc:
        with tc.tile_pool(name="const", bufs=1) as cpool, \
             tc.tile_pool(name="store", bufs=1) as spool, \
             tc.tile_pool(name="g", bufs=4) as gpool, \
             tc.tile_pool(name="work", bufs=3) as wpool, \
             tc.tile_pool(name="permp", bufs=4) as ppool, \
             tc.tile_pool(name="psA", bufs=3, space="PSUM") as psA, \
             tc.tile_pool(name="psB", bufs=2, space="PSUM") as psB, \
             tc.tile_pool(name="psC", bufs=2, space="PSUM") as psC, \
             tc.tile_pool(name="dram", bufs=1, space="DRAM") as dpool:

            ident32 = cpool.tile([P, P], f32)
            make_identity(nc, ident32[:])
            ident_bf = cpool.tile([P, P], bf)
            nc.vector.tensor_copy(out=ident_bf[:], in_=ident32[:])

            Wbf = []
            for i in range(2):
                wt = cpool.tile([P, D], f32, name=f"w32_{i}")
                nc.sync.dma_start(out=wt[:], in_=W_ext[i][:])
                wb = cpool.tile([P, D], bf, name=f"wbf_{i}")
                nc.vector.tensor_copy(out=wb[:], in_=wt[:])
                Wbf.append(wb)

            vecs = {}
            for nm in vecs_ext:
                vt = cpool.tile([P, D], f32, name=f"vec_{nm}")
                nc.sync.dma_start(out=vt[:], in_=vecs_ext[nm][:])
                vecs[nm] = vt

            deg_t = cpool.tile([P, TILES], f32)
            nc.sync.dma_start(out=deg_t[:], in_=deg_ext[:])
            sq = cpool.tile([P, TILES], f32)
            nc.scalar.activation(out=sq[:], in_=deg_t[:], func=AF.Sqrt)
            dinv = cpool.tile([P, TILES], f32)
            nc.vector.reciprocal(dinv[:], sq[:])

            eps_t = cpool.tile([P, 1], f32)
            nc.vector.memset(eps_t[:], float(LN_EPS))
            # per-partition mask: 1.0 for real lanes of the last tile, 0.0 for dummies
            ndum = SHARD - 6250
            dmask = cpool.tile([P, 1], f32)
            nc.vector.memset(dmask[:], 1.0)
            nc.gpsimd.affine_select(
                out=dmask[:], in_=dmask[:], pattern=[[0, 1]],
                compare_op=OP.is_ge, fill=0.0,
                base=P - ndum - 1, channel_multiplier=-1)
            idx_t = cpool.tile([P, totcols], mybir.dt.int16)
            nc.sync.dma_start(out=idx_t[:], in_=idx_ext[:])
            tgt_t = cpool.tile([P, pl.TB], f32)
            nc.sync.dma_start(out=tgt_t[:], in_=tgt_ext[:])
            iota_i = cpool.tile([P, P], mybir.dt.int32)
            nc.gpsimd.iota(iota_i[:], pattern=[[1, P]], base=0, channel_multiplier=0)
            iota_row = cpool.tile([P, P], f32)
            nc.vector.tensor_copy(out=iota_row[:], in_=iota_i[:])

            x_store = spool.tile([P, TILES, D], f32)
            nc.sync.dma_start(out=x_store[:],
                              in_=x_ext[:].rearrange("(t l) f -> l t f", t=TILES))
            xs_store = spool.tile([P, TILES, D], bf)
            for t in range(TILES):
                nc.scalar.activation(out=xs_store[:, t, :], in_=x_store[:, t, :],
                                     func=AF.Identity, scale=dinv[:, t:t + 1])

            dinvm = cpool.tile([P, 1], f32)
            nc.vector.tensor_scalar(out=dinvm[:], in0=dinv[:, TILES - 1:TILES],
                                    scalar1=dmask[:, 0:1], scalar2=None, op0=OP.mult)
            h1g_store = spool.tile([P, TILES, D], bf)
            H1sb = spool.tile([P, TILES, D], bf)

            cc_in = [dpool.tile([SHARD, D], bf, name=f"ccin{i}") for i in range(2)]
            cc_out = [dpool.tile([NC * SHARD, D], bf, name=f"ccout{i}",
                                 addr_space="Shared") for i in range(2)]

            NBMAX = int(pl.NBLK.max())

            def finish_h0_tile(layer, t, ps, started):
                selfstore = xs_store if layer == 0 else h1g_store
                nc.tensor.matmul(out=ps[:], lhsT=ident_bf[:],
                                 rhs=selfstore[:, t, :], start=not started, stop=False)
                bi0 = int(pl.NBLK[:t].sum())
                nb = len(pl.block_t1[t])
                assert nb >= 1
                pb = ppool.tile([P, NBMAX, P], bf, tag="pb", name=f"pb_{layer}_{t}")
                for j in range(nb):
                    nc.vector.tensor_scalar(out=pb[:, j, :], in0=iota_row[:],
                                            scalar1=tgt_t[:, bi0 + j:bi0 + j + 1],
                                            scalar2=None, op0=OP.is_equal)
                for j, t1 in enumerate(pl.block_t1[t]):
                    nc.tensor.matmul(out=ps[:], lhsT=pb[:, j, :], rhs=H1sb[:, t1, :],
                                     start=False, stop=(j == nb - 1))

                # evict agg -> bf16 (no dinv scale: LN is scale-invariant when b==0)
                b_triv, g_triv, be_triv = triv[layer]
                s_agg = wpool.tile([P, D], bf, tag="sagg", name=f"sagg_{layer}_{t}")
                if b_triv:
                    nc.scalar.activation(out=s_agg[:], in_=ps[:], func=AF.Identity)
                else:
                    nc.scalar.activation(out=s_agg[:], in_=ps[:], func=AF.Identity,
                                         scale=dinv[:, t:t + 1])
                psT = psB.tile([P, D], bf, space="PSUM", tag="tr",
                               name=f"psT_{layer}_{t}")
                nc.tensor.transpose(out=psT[:], in_=s_agg[:], identity=ident_bf[:])
                s_aggT = wpool.tile([P, D], bf, tag="saggT", name=f"saggT_{layer}_{t}")
                nc.vector.tensor_copy(out=s_aggT[:], in_=psT[:])
                convp = psC.tile([P, D], f32, space="PSUM", tag="conv",
                                 name=f"conv_{layer}_{t}")
                nc.tensor.matmul(out=convp[:], lhsT=s_aggT[:], rhs=Wbf[layer][:],
                                 start=True, stop=True)

                if b_triv:
                    cb_ap = convp[:]
                else:
                    bv = vecs["b1" if layer == 0 else "b2"]
                    cb = wpool.tile([P, D], f32, tag="cb", name=f"cb_{layer}_{t}")
                    nc.vector.tensor_tensor(out=cb[:], in0=convp[:], in1=bv[:], op=OP.add)
                    cb_ap = cb[:]
                scr = wpool.tile([P, D], f32, tag="scr", name=f"scr_{layer}_{t}")
                negmu = wpool.tile([P, 1], f32, tag="negmu", name=f"negmu_{layer}_{t}")
                nc.scalar.activation(out=scr[:], in_=cb_ap, func=AF.Identity,
                                     scale=-1.0 / D, accum_out=negmu[:])
                ctr = wpool.tile([P, D], f32, tag="ctr", name=f"ctr_{layer}_{t}")
                nc.scalar.activation(out=ctr[:], in_=cb_ap, func=AF.Identity,
                                     bias=negmu[:, 0:1])
                sqs = wpool.tile([P, D], f32, tag="sqs", name=f"sqs_{layer}_{t}")
                var_raw = wpool.tile([P, 1], f32, tag="varr", name=f"varr_{layer}_{t}")
                nc.scalar.activation(out=sqs[:], in_=ctr[:], func=AF.Square,
                                     scale=float(1.0 / np.sqrt(D)),
                                     accum_out=var_raw[:])
                std = wpool.tile([P, 1], f32, tag="std", name=f"std_{layer}_{t}")
                nc.scalar.activation(out=std[:], in_=var_raw[:], func=AF.Sqrt,
                                     bias=eps_t[:, 0:1])
                rstd = wpool.tile([P, 1], f32, tag="rstd", name=f"rstd_{layer}_{t}")
                nc.vector.reciprocal(rstd[:], std[:])

                if not (g_triv and be_triv):
                    gv = vecs["g1" if layer == 0 else "g2"]
                    bev = vecs["beta1" if layer == 0 else "beta2"]
                    o1 = wpool.tile([P, D], f32, tag="o1", name=f"o1_{layer}_{t}")
                    nc.scalar.activation(out=o1[:], in_=ctr[:], func=AF.Identity,
                                         scale=rstd[:, 0:1])
                    o2 = wpool.tile([P, D], f32, tag="o2", name=f"o2_{layer}_{t}")
                    nc.vector.tensor_tensor(out=o2[:], in0=o1[:], in1=gv[:], op=OP.mult)
                    o3 = wpool.tile([P, D], f32, tag="o3", name=f"o3_{layer}_{t}")
                    nc.vector.tensor_tensor(out=o3[:], in0=o2[:], in1=bev[:], op=OP.add)
                    if layer == 0:
                        o4 = wpool.tile([P, D], f32, tag="o4", name=f"o4_{t}")
                        nc.scalar.activation(out=o4[:], in_=o3[:], func=AF.Relu)
                        dcol = dinvm[:, 0:1] if t == TILES - 1 else dinv[:, t:t + 1]
                        nc.vector.tensor_scalar(out=h1g_store[:, t, :], in0=o4[:],
                                                scalar1=dcol, scalar2=None,
                                                op0=OP.mult)
                    else:
                        nc.sync.dma_start(out=out_ext[t * P:(t + 1) * P, :], in_=o3[:])
                else:
                    if layer == 0:
                        dcol = dinvm[:, 0:1] if t == TILES - 1 else dinv[:, t:t + 1]
                        rsd = wpool.tile([P, 1], f32, tag="rsd", name=f"rsd_{t}")
                        nc.vector.tensor_scalar(out=rsd[:], in0=rstd[:],
                                                scalar1=dcol, scalar2=None,
                                                op0=OP.mult)
                        nc.scalar.activation(out=h1g_store[:, t, :], in_=ctr[:],
                                             func=AF.Relu, scale=rsd[:, 0:1])
                    else:
                        o1 = wpool.tile([P, D], f32, tag="o1", name=f"o1_{layer}_{t}")
                        nc.scalar.activation(out=o1[:], in_=ctr[:], func=AF.Identity,
                                             scale=rstd[:, 0:1])
                        nc.sync.dma_start(out=out_ext[t * P:(t + 1) * P, :], in_=o1[:])

            def run_layer(layer):
                selfstore = xs_store if layer == 0 else h1g_store
                for t0 in range(0, TILES, 7):
                    t1b = min(t0 + 7, TILES)
                    nc.sync.dma_start(
                        out=cc_in[layer][t0 * P:t1b * P, :].rearrange(
                            "(t l) f -> l t f", t=t1b - t0),
                        in_=selfstore[:, t0:t1b, :])
                nc.gpsimd.collective_compute(
                    "AllGather", OP.bypass,
                    replica_groups=[list(range(NC))],
                    ins=[cc_in[layer][:].opt()],
                    outs=[cc_out[layer][:].opt()],
                )
                table = cc_out[layer]
                nc.vector.memset(H1sb[:], 0.0)

                open_ps = {}                 # (h, t) -> (ps tile, started)
                if True:
                    for gcall, (h, gi) in enumerate(pl.sched):
                        half_ap = table[HALF_ROWS:, :] if h == 1 else table[:HALF_ROWS, :]
                        grp = pl.groups[h][gi]
                        c0, c1 = pl.col_ranges[h][gi]
                        nch = sum(r for (_, r, _) in grp)
                        gbuf = gpool.tile([P, GBUF_CHUNKS, D], bf, tag="g",
                                          name=f"g_{layer}_{h}_{gi}")
                        nc.gpsimd.dma_gather(
                            out_ap=gbuf[:, :nch, :],
                            in_ap=half_ap,
                            idxs_ap=idx_t[:, c0:c1],
                            num_idxs=nch * P,
                            num_idxs_reg=nch * P,
                            elem_size=D,
                            single_packet=False,
                            queue_num=gcall % 4,
                        )
                        off = 0
                        for (t, r, partial) in grp:
                            key = (h, t)
                            if key in open_ps:
                                ps, started = open_ps.pop(key)
                            else:
                                ps = psA.tile([P, D], f32, space="PSUM", tag="agg",
                                              name=f"ps_{layer}_{h}_{t}")
                                started = False
                            for ri in range(r):
                                last = (not partial) and (h == 1) and (ri == r - 1)
                                nc.tensor.matmul(out=ps[:], lhsT=ident_bf[:],
                                                 rhs=gbuf[:, off + ri, :],
                                                 start=not started, stop=last)
                                started = True
                            off += r
                            if partial:
                                open_ps[key] = (ps, started)
                            elif h == 1:
                                nc.scalar.activation(out=H1sb[:, t, :], in_=ps[:],
                                                     func=AF.Identity)
                            else:
                                finish_h0_tile(layer, t, ps, started)
                covered = set(t for grp in pl.groups[0] for (t, _, pa) in grp if not pa)
                for t in range(TILES):
                    if t not in covered:
                        ps = psA.tile([P, D], f32, space="PSUM", tag="agg",
                                      name=f"ps_{layer}_0z_{t}")
                        finish_h0_tile(layer, t, ps, False)

            run_layer(0)
            run_layer(1)

    nc.compile()
    return nc


# ----------------------------------------------------------------------------
# Entry point
# ----------------------------------------------------------------------------

_last_result = None


def kernel(**inputs) -> np.ndarray:
    edge_index = np.asarray(inputs["edge_index"])
    pl = build_plan(edge_index)

    if os.environ.get("KERNEL_EMULATE") == "1":
        return emulate(pl, inputs)

    from concourse.bass_utils import run_bass_kernel_spmd
    triv = []
    for i in (1, 2):
        triv.append((
            not np.any(np.asarray(inputs[f"b{i}"])),
            np.all(np.asarray(inputs[f"g{i}"]) == 1.0),
            not np.any(np.asarray(inputs[f"beta{i}"])),
        ))
    nc = build_bass(pl, triv)

    x = np.asarray(inputs["x"], dtype=np.float32)
    in_maps = []
    for c in range(NC):
        deg_t = np.ones((P, TILES), dtype=np.float32)
        xp = np.zeros((SHARD, D), dtype=np.float32)
        valid = pl.node_at[c] >= 0
        pidx = np.arange(SHARD)
        deg_t[pidx[valid] % P, pidx[valid] // P] = pl.deg[pl.node_at[c][valid]]
        xp[valid] = x[pl.node_at[c][valid]]
        m = {
            "x": xp,
            "deg": deg_t,
            "idx": pl.idx_in[c],
            "tgt": pl.tgt_in[c],
            "W1": np.asarray(inputs["W1"], np.float32),
            "W2": np.asarray(inputs["W2"], np.float32),
        }
        for nm in ["b1", "g1", "beta1", "b2", "g2", "beta2"]:
            m[nm] = np.tile(np.asarray(inputs[nm], np.float32)[None, :], (P, 1))
        in_maps.append(m)

    kw = {}
    if os.environ.get("KERNEL_TRACE") == "1":
        kw = dict(trace=True, trace_cores=[0])
    res = run_bass_kernel_spmd(nc, in_maps, core_ids=list(range(NC)), **kw)
    global _last_result
    _last_result = res

    out = np.zeros((N, D), dtype=np.float32)
    for c in range(NC):
        o = np.asarray(res.results[c]["out"], dtype=np.float32)
        valid = pl.node_at[c] >= 0
        out[pl.node_at[c][valid]] = o[valid]
    return out



# revision 9
# speedup vs baseline: 2.5922x; 1.8033x over previous
"""2-layer GCN (GCNConv + LayerNorm + ReLU + GCNConv + LayerNorm) on 8 TRN2 NeuronCores.

v2 design:
  - Nodes degree-sorted, dealt round-robin to 8 cores; 6250 dst nodes/core
    (padded to 6272 = 49 tiles of 128 lanes). Single storage tiling (no
    per-half re-tiling): tile/lane of a node is the same for gather targets
    and storage.
  - Global gather table layout is tile-group-major: 7 groups of [8,8,8,8,8,8,1]
    tiles; within a group rows are (core, tile, lane). Layer-1 table (dinv-
    scaled x, bf16) is built on host and passed as an input parameter; layer-2
    table is assembled with 7 chunked AllGathers issued as tile groups finish,
    overlapping the layer-1 gather stream.
  - Gather uses SIGNED int16 indices with the DMA base planted at table row
    32768: idx = row - 32768 spans [-32768, 17407], covering all 50176 rows in
    ONE stream (the Q7 ucode sign-extends idxs and IVP_MULUSAN multiplies them
    signed). This removes the H0/H1 split, the fold permutation matmuls, and
    all IS_EQ one-hot building. Only trailing-negative idxs are trimmed by the
    ucode, so each gather call must END on a non-negative idx — the planner
    reorders each call's last chunk to end on a pad (pads point at a
    guaranteed-zero dummy row, idx +17407).
  - Gather calls are spread round-robin over 4 SWDGE queues; each queue
    activates a different Q7 core pair, so descriptor generation for 4 calls
    proceeds in parallel.
  - Aggregation accumulates TRANSPOSED: matmul(lhsT=chunk, rhs=identity)
    gives psum[f, d], so the W matmul (lhsT=aggT, rhs=W) directly yields
    row-major conv[d, f'] — no per-tile PE transpose + copy.
  - b==0 in this problem, so the dst-side dinv scale is absorbed by
    LayerNorm's scale invariance; layer-1 outputs are re-scaled by dinv (and
    dummy lanes zeroed) when stored as next-layer table rows.
"""
import os
import numpy as np
import ml_dtypes

N = 50000
E = 600000
D = 128
NC = 8
P = 128
SHARD = 6272            # 49 * 128
TILES = 49
GROUP_SZ = [8, 8, 8, 8, 8, 8, 1]      # tiles per AllGather group
BASE_ROW = 32768        # gather base row (idx 0 lands here)
PAD_ROW = 50175         # (core 7, tile 48, lane 127) -> dummy zero row
LN_EPS = 1e-5
GB = 32                 # chunks (128 edges each) per dma_gather call
NQ = 4                  # SWDGE queues

bf16 = ml_dtypes.bfloat16


# ----------------------------------------------------------------------------
# Host-side planning (index-only preprocessing)
# ----------------------------------------------------------------------------

class Plan:
    pass


def _row_of(core, tile, lane):
    """Table row for (core, tile, lane) in tile-group-major layout."""
    g = np.minimum(tile // 8, 6)
    gbase = np.asarray([0, 8192, 16384, 24576, 32768, 40960, 49152])[g]
    szg = np.asarray(GROUP_SZ)[g]
    return gbase + core * szg * P + (tile - 8 * g) * P + lane


def build_plan(edge_index: np.ndarray) -> Plan:
    pl = Plan()
    src = edge_index[0].astype(np.int64)
    dst = edge_index[1].astype(np.int64)

    deg = np.bincount(dst, minlength=N) + 1          # incl. mandatory self-loop
    order = np.argsort(-deg, kind="stable")          # global degree desc
    core_of = np.empty(N, dtype=np.int64)
    core_of[order] = np.arange(N) % NC               # deal round-robin
    pos = np.empty(N, dtype=np.int64)
    for c in range(NC):
        shard = order[c::NC]                          # 6250 nodes, deg desc
        pos[shard] = np.arange(len(shard))
    tile_of = pos // P
    lane_of = pos % P
    row = _row_of(core_of, tile_of, lane_of)
    idx16 = row - BASE_ROW                           # signed, [-32768, 17407]

    node_at = np.full((NC, SHARD), -1, dtype=np.int64)
    for c in range(NC):
        shard = order[c::NC]
        node_at[c, pos[shard]] = shard
    pl.node_at = node_at
    pl.deg = deg
    pl.row = row

    deg_in = deg - 1
    # per-tile rounds, uniform over cores (SPMD identical programs)
    m = np.zeros(NC * TILES, dtype=np.int64)
    np.maximum.at(m, core_of * TILES + tile_of, deg_in)
    R_uni = m.reshape(NC, TILES).max(axis=0)
    assert R_uni.min() >= 1
    pl.R_uni = R_uni
    chunk_base = np.zeros(TILES + 1, dtype=np.int64)
    chunk_base[1:] = np.cumsum(R_uni)
    n_chunks = int(chunk_base[-1])
    pl.chunk_base = chunk_base
    pl.n_chunks = n_chunks
    tile_of_chunk = np.repeat(np.arange(TILES), R_uni)

    # round index for each edge: rank among edges with same dst
    eorder = np.argsort(dst, kind="stable")
    sd = dst[eorder]
    starts = np.r_[0, np.flatnonzero(sd[1:] != sd[:-1]) + 1]
    group_of = np.zeros(E, dtype=np.int64)
    group_of[starts[1:]] = 1
    group_of = np.cumsum(group_of)
    rounds_sorted = np.arange(E) - starts[group_of]
    rounds = np.empty(E, dtype=np.int64)
    rounds[eorder] = rounds_sorted

    # slot arrays per core: [n_chunks*128] of signed idx values (pad -> zero row)
    PAD_IDX = PAD_ROW - BASE_ROW
    slots = [np.full(n_chunks * P, PAD_IDX, dtype=np.int64) for _ in range(NC)]
    e_core = core_of[dst]
    e_slot = (chunk_base[tile_of[dst]] + rounds) * P + lane_of[dst]
    e_val = idx16[src]
    for c in range(NC):
        mm = e_core == c
        slots[c][e_slot[mm]] = e_val[mm]

    # calls: consecutive chunks, <= GB each. The ucode trims TRAILING negative
    # idxs, so each call's very last slot (lane 127 of its final chunk) must be
    # non-negative in EVERY core. Round order within a (tile, lane) is free per
    # core, so swap a pad (positive) or positive-edge round into that slot.
    calls = []                                       # list of lists of chunk ids
    for c0 in range(0, n_chunks, GB):
        chunks = list(range(c0, min(c0 + GB, n_chunks)))
        final = None
        for cand in reversed(chunks):
            t = int(tile_of_chunk[cand])
            r = cand - int(chunk_base[t])
            rounds_sl = [(int(chunk_base[t]) + rr) * P + 127
                         for rr in range(int(R_uni[t]))]
            swaps = []                               # (core, slot_a, slot_b)
            ok = True
            for c in range(NC):
                sl = (int(chunk_base[t]) + r) * P + 127
                if slots[c][sl] >= 0:
                    continue                         # already safe
                cand_sl = [s for s in rounds_sl if slots[c][s] >= 0]
                if not cand_sl:
                    ok = False
                    break
                swaps.append((c, sl, cand_sl[-1]))
                cand_sl.pop()
            if ok:
                final = cand
                for c, a, bsl in swaps:
                    slots[c][a], slots[c][bsl] = slots[c][bsl], slots[c][a]
                break
        assert final is not None, f"no fixable final chunk in call at {c0}"
        chunks.remove(final)
        chunks.append(final)
        calls.append(chunks)
    pl.calls = calls
    pl.tile_of_chunk = tile_of_chunk

    def wrap(flat):                                  # [num] -> [128, num//16]
        num = len(flat)
        w = np.zeros((16, num // 16), dtype=np.int16)
        w[np.arange(num) % 16, np.arange(num) // 16] = flat.astype(np.int16)
        return np.tile(w, (8, 1))

    idx_in = []
    col_ranges = []
    for c in range(NC):
        parts = []
        col = 0
        for chunks in calls:
            seg = np.concatenate([slots[c][ch * P:(ch + 1) * P]
                                  for ch in chunks])
            parts.append(wrap(seg))
            if c == 0:
                col_ranges.append((col, col + len(seg) // 16))
            col += len(seg) // 16
        idx_in.append(np.concatenate(parts, axis=1))
    pl.idx_in = idx_in
    pl.col_ranges = col_ranges
    return pl


def host_inputs(pl, inputs):
    """Per-core input tensors (elementwise/reindex preprocessing only)."""
    x = np.asarray(inputs["x"], dtype=np.float32)
    deg = pl.deg
    dinv_n = 1.0 / np.sqrt(deg.astype(np.float64))

    # global layer-1 table: dinv-scaled x rows in table layout, bf16
    tab0 = np.zeros((50176, D), dtype=bf16)
    valid = pl.node_at >= 0
    for c in range(NC):
        nodes = pl.node_at[c][valid[c]]
        rows = pl.row[nodes]
        tab0[rows] = (x[nodes] * dinv_n[nodes][:, None]).astype(bf16)

    per_core = []
    for c in range(NC):
        nodes = pl.node_at[c]
        v = nodes >= 0
        pidx = np.arange(SHARD)
        # local rows [lane, tile, feat] (same values as tab0 own-shard rows)
        xloc = np.zeros((P, TILES, D), dtype=bf16)
        xloc[pidx[v] % P, pidx[v] // P] = (
            x[nodes[v]] * dinv_n[nodes[v]][:, None]).astype(bf16)
        dinv_t = np.ones((P, TILES), dtype=np.float32)
        dinv_t[pidx[v] % P, pidx[v] // P] = dinv_n[nodes[v]].astype(np.float32)
        dinvm = dinv_t[:, TILES - 1:TILES].copy()
        dinvm[pidx[~v] % P] = 0.0                     # zero dummy lanes (tile 48)
        m = {
            "tab0": tab0,
            "xloc": xloc.reshape(P, TILES * D),
            "dinv": dinv_t,
            "dinvm": dinvm,
            "idx": pl.idx_in[c],
            "W1": np.asarray(inputs["W1"], np.float32),
            "W2": np.asarray(inputs["W2"], np.float32),
        }
        for nm in ["b1", "g1", "beta1", "b2", "g2", "beta2"]:
            m[nm] = np.tile(np.asarray(inputs[nm], np.float32)[None, :], (P, 1))
        per_core.append(m)
    return per_core


# ----------------------------------------------------------------------------
# Numpy emulation of the device program (for validating the plan quickly)
# ----------------------------------------------------------------------------

def emulate2(pl, inputs):
    """Faithful emulation consuming the WRAPPED idx tensors exactly as the
    device would (validates slot packing, call reordering, signed idxs)."""
    W = [np.asarray(inputs["W1"], np.float32), np.asarray(inputs["W2"], np.float32)]
    b = [np.asarray(inputs["b1"], np.float32), np.asarray(inputs["b2"], np.float32)]
    g = [np.asarray(inputs["g1"], np.float32), np.asarray(inputs["g2"], np.float32)]
    be = [np.asarray(inputs["beta1"], np.float32), np.asarray(inputs["beta2"], np.float32)]
    per_core = host_inputs(pl, inputs)

    def tobf(a):
        return a.astype(bf16).astype(np.float32)

    tab = np.asarray(per_core[0]["tab0"]).astype(np.float32)   # layer-1 table
    h1g_all = [None] * NC
    out_full = np.zeros((N, D), dtype=np.float32)

    for layer in range(2):
        ntab = np.zeros((50176, D), dtype=np.float32)
        for c in range(NC):
            xs = np.asarray(per_core[c]["xloc"], np.float32).reshape(P, TILES, D)
            if layer == 1:
                xs = h1g_all[c]
            dinv_t = np.asarray(per_core[c]["dinv"], np.float32)
            dinvm = np.asarray(per_core[c]["dinvm"], np.float32)

            psT = {}                                   # tile -> [D, P] accum
            remaining = {t: int(pl.R_uni[t]) for t in range(TILES)}
            for ci, chunks in enumerate(pl.calls):
                c0, c1 = pl.col_ranges[ci]
                wrapped = pl.idx_in[c][:16, c0:c1].astype(np.int64)
                num = (c1 - c0) * 16
                flat = np.empty(num, dtype=np.int64)
                ar = np.arange(num)
                flat[ar] = wrapped[ar % 16, ar // 16]
                # emulate ucode trailing-negative trim
                nn = num
                while nn > 0 and flat[nn - 1] < 0:
                    nn -= 1
                assert nn == num, f"call {ci} would be trimmed! (core {c})"
                rows = flat + BASE_ROW
                gath = (tab if layer == 0 else ntab_prev)[rows].reshape(-1, P, D)
                for i, ch in enumerate(chunks):
                    t = int(pl.tile_of_chunk[ch])
                    if t not in psT:
                        psT[t] = xs[:, t, :].T.copy()   # self-loop opens tile
                    psT[t] += gath[i].T
                    remaining[t] -= 1
                    if remaining[t] == 0:
                        aggT = tobf(psT.pop(t))          # bf16 eviction
                        conv = aggT.T @ tobf(W[layer])   # [d, f']
                        cb = conv + b[layer][None, :]
                        mu = cb.mean(axis=1, keepdims=True)
                        ctr = cb - mu
                        var = (ctr ** 2).mean(axis=1, keepdims=True)
                        o = ctr / np.sqrt(var + LN_EPS) * g[layer][None, :] + be[layer][None, :]
                        if layer == 0:
                            o = np.maximum(o, 0.0)
                            dcol = dinvm[:, 0] if t == TILES - 1 else dinv_t[:, t]
                            h1g_all[c] = h1g_all[c] if h1g_all[c] is not None else \
                                np.zeros((P, TILES, D), dtype=np.float32)
                            h1g_all[c][:, t, :] = tobf(o * dcol[:, None])
                        else:
                            outs = o
                            pidx = np.arange(t * P, (t + 1) * P)
                            nodes = pl.node_at[c][pidx]
                            v = nodes >= 0
                            out_full[nodes[v]] = o[v]
            assert not psT, f"unclosed tiles {list(psT)} core {c}"
            if layer == 0:
                # core's h1g rows -> next-layer table (AllGather emulation)
                for t in range(TILES):
                    rows = _row_of(c, t, np.arange(P))
                    ntab[rows] = h1g_all[c][:, t, :]
        ntab_prev = ntab
    return out_full


# ----------------------------------------------------------------------------
# Bass kernel
# ----------------------------------------------------------------------------

def build_bass(pl, triv):
    import concourse.bacc as bacc
    import concourse.mybir as mybir
    import concourse.tile as tile
    from concourse.masks import make_identity

    f32 = mybir.dt.float32
    bf = mybir.dt.bfloat16
    AF = mybir.ActivationFunctionType
    OP = mybir.AluOpType

    nc = bacc.Bacc("TRN2", target_bir_lowering=False, debug=False, num_devices=NC,
                   num_swdge_queues=NQ, dynamic_dma_scratch_size=16384)

    tab0_ext = nc.declare_dram_parameter("tab0", [50176, D], bf, isOutput=False)
    xloc_ext = nc.declare_dram_parameter("xloc", [P, TILES * D], bf, isOutput=False)
    dinv_ext = nc.declare_dram_parameter("dinv", [P, TILES], f32, isOutput=False)
    dinvm_ext = nc.declare_dram_parameter("dinvm", [P, 1], f32, isOutput=False)
    totcols = pl.idx_in[0].shape[1]
    idx_ext = nc.declare_dram_parameter("idx", [P, totcols], mybir.dt.int16, isOutput=False)
    W_ext = [nc.declare_dram_parameter(f"W{i+1}", [D, D], f32, isOutput=False) for i in range(2)]
    vecs_ext = {}
    for nm in ["b1", "g1", "beta1", "b2", "g2", "beta2"]:
        vecs_ext[nm] = nc.declare_dram_parameter(nm, [P, D], f32, isOutput=False)
    out_ext = nc.declare_dram_parameter("out", [SHARD, D], f32, isOutput=True)

    # group tile ranges
    gstart = [0, 8, 16, 24, 32, 40, 48]
    grows_in = [(gs * P, (gs + sz) * P) for gs, sz in zip(gstart, GROUP_SZ)]
    grows_out = []
    rb = 0
    for sz in GROUP_SZ:
        grows_out.append((rb, rb + sz * P * NC))
        rb += sz * P * NC

    with tile.TileContext(nc) as tc:
        with tc.tile_pool(name="const", bufs=1) as cpool, \
             tc.tile_pool(name="store", bufs=1) as spool, \
             tc.tile_pool(name="g", bufs=8) as gpool, \
             tc.tile_pool(name="work", bufs=3) as wpool, \
             tc.tile_pool(name="psA", bufs=5, space="PSUM") as psA, \
             tc.tile_pool(name="psC", bufs=3, space="PSUM") as psC, \
             tc.tile_pool(name="dram", bufs=1, space="DRAM") as dpool:

            ident32 = cpool.tile([P, P], f32)
            make_identity(nc, ident32[:])
            ident_bf = cpool.tile([P, P], bf)
            nc.vector.tensor_copy(out=ident_bf[:], in_=ident32[:])

            idx_t = cpool.tile([P, totcols], mybir.dt.int16)
            nc.sync.dma_start(out=idx_t[:], in_=idx_ext[:])

            xs_store = spool.tile([P, TILES, D], bf)
            nc.sync.dma_start(
                out=xs_store[:].rearrange("l t f -> l (t f)"), in_=xloc_ext[:])

            Wbf = []
            for i in range(2):
                wt = cpool.tile([P, D], f32, name=f"w32_{i}")
                nc.sync.dma_start(out=wt[:], in_=W_ext[i][:])
                wb = cpool.tile([P, D], bf, name=f"wbf_{i}")
                nc.vector.tensor_copy(out=wb[:], in_=wt[:])
                Wbf.append(wb)

            vecs = {}
            for nm in vecs_ext:
                vt = cpool.tile([P, D], f32, name=f"vec_{nm}")
                nc.sync.dma_start(out=vt[:], in_=vecs_ext[nm][:])
                vecs[nm] = vt

            dinv = cpool.tile([P, TILES], f32)
            nc.sync.dma_start(out=dinv[:], in_=dinv_ext[:])
            dinvm = cpool.tile([P, 1], f32)
            nc.sync.dma_start(out=dinvm[:], in_=dinvm_ext[:])
            eps_t = cpool.tile([P, 1], f32)
            nc.vector.memset(eps_t[:], float(LN_EPS))

            h1g_store = spool.tile([P, TILES, D], bf)

            cc_in = dpool.tile([SHARD, D], bf, name="ccin")
            cc_ag = [dpool.tile([GROUP_SZ[g] * P * NC, D], bf, name=f"ccag{g}",
                                addr_space="Shared") for g in range(7)]
            table2 = dpool.tile([NC * SHARD, D], bf, name="table2")

            def finish_tile(layer, t, psT):
                b_triv, g_triv, be_triv = triv[layer]
                s_aggT = wpool.tile([P, D], bf, tag="saggT", name=f"saT_{layer}_{t}")
                nc.scalar.activation(out=s_aggT[:], in_=psT[:], func=AF.Identity)
                convp = psC.tile([P, D], f32, space="PSUM", tag="conv",
                                 name=f"conv_{layer}_{t}")
                nc.tensor.matmul(out=convp[:], lhsT=s_aggT[:], rhs=Wbf[layer][:],
                                 start=True, stop=True)

                if b_triv:
                    cb_ap = convp[:]
                else:
                    # restore dst-side dinv scale before bias (LN no longer absorbs it)
                    sc = wpool.tile([P, D], f32, tag="sc", name=f"sc_{layer}_{t}")
                    dcol = dinvm[:, 0:1] if t == TILES - 1 else dinv[:, t:t + 1]
                    nc.scalar.activation(out=sc[:], in_=convp[:], func=AF.Identity,
                                         scale=dcol)
                    bv = vecs["b1" if layer == 0 else "b2"]
                    cb = wpool.tile([P, D], f32, tag="cb", name=f"cb_{layer}_{t}")
                    nc.vector.tensor_tensor(out=cb[:], in0=sc[:], in1=bv[:], op=OP.add)
                    cb_ap = cb[:]
                scr = wpool.tile([P, D], f32, tag="scr", name=f"scr_{layer}_{t}")
                negmu = wpool.tile([P, 1], f32, tag="negmu", name=f"negmu_{layer}_{t}")
                nc.scalar.activation(out=scr[:], in_=cb_ap, func=AF.Identity,
                                     scale=-1.0 / D, accum_out=negmu[:])
                ctr = wpool.tile([P, D], f32, tag="ctr", name=f"ctr_{layer}_{t}")
                nc.scalar.activation(out=ctr[:], in_=cb_ap, func=AF.Identity,
                                     bias=negmu[:, 0:1])
                sqs = wpool.tile([P, D], f32, tag="sqs", name=f"sqs_{layer}_{t}")
                var_raw = wpool.tile([P, 1], f32, tag="varr", name=f"varr_{layer}_{t}")
                nc.scalar.activation(out=sqs[:], in_=ctr[:], func=AF.Square,
                                     scale=float(1.0 / np.sqrt(D)),
                                     accum_out=var_raw[:])
                std = wpool.tile([P, 1], f32, tag="std", name=f"std_{layer}_{t}")
                nc.scalar.activation(out=std[:], in_=var_raw[:], func=AF.Sqrt,
                                     bias=eps_t[:, 0:1])
                rstd = wpool.tile([P, 1], f32, tag="rstd", name=f"rstd_{layer}_{t}")
                nc.vector.reciprocal(rstd[:], std[:])

                if not (g_triv and be_triv):
                    gv = vecs["g1" if layer == 0 else "g2"]
                    bev = vecs["beta1" if layer == 0 else "beta2"]
                    o1 = wpool.tile([P, D], f32, tag="o1", name=f"o1_{layer}_{t}")
                    nc.scalar.activation(out=o1[:], in_=ctr[:], func=AF.Identity,
                                         scale=rstd[:, 0:1])
                    o2 = wpool.tile([P, D], f32, tag="o2", name=f"o2_{layer}_{t}")
                    nc.vector.tensor_tensor(out=o2[:], in0=o1[:], in1=gv[:], op=OP.mult)
                    o3 = wpool.tile([P, D], f32, tag="o3", name=f"o3_{layer}_{t}")
                    nc.vector.tensor_tensor(out=o3[:], in0=o2[:], in1=bev[:], op=OP.add)
                    if layer == 0:
                        o4 = wpool.tile([P, D], f32, tag="o4", name=f"o4_{t}")
                        nc.scalar.activation(out=o4[:], in_=o3[:], func=AF.Relu)
                        dcol = dinvm[:, 0:1] if t == TILES - 1 else dinv[:, t:t + 1]
                        nc.vector.tensor_scalar(out=h1g_store[:, t, :], in0=o4[:],
                                                scalar1=dcol, scalar2=None,
                                                op0=OP.mult)
                    else:
                        nc.sync.dma_start(out=out_ext[t * P:(t + 1) * P, :], in_=o3[:])
                else:
                    if layer == 0:
                        dcol = dinvm[:, 0:1] if t == TILES - 1 else dinv[:, t:t + 1]
                        rsd = wpool.tile([P, 1], f32, tag="rsd", name=f"rsd_{t}")
                        nc.vector.tensor_scalar(out=rsd[:], in0=rstd[:],
                                                scalar1=dcol, scalar2=None,
                                                op0=OP.mult)
                        nc.scalar.activation(out=h1g_store[:, t, :], in_=ctr[:],
                                             func=AF.Relu, scale=rsd[:, 0:1])
                    else:
                        o1 = wpool.tile([P, D], f32, tag="o1", name=f"o1_{layer}_{t}")
                        nc.scalar.activation(out=o1[:], in_=ctr[:], func=AF.Identity,
                                             scale=rstd[:, 0:1])
                        nc.sync.dma_start(out=out_ext[t * P:(t + 1) * P, :], in_=o1[:])

            # --- layer runner with deferred AllGather firing ---
            def run_layer2(layer):
                selfstore = xs_store if layer == 0 else h1g_store
                table_ap = (tab0_ext if layer == 0 else table2)[BASE_ROW:, :]
                open_ps = {}
                remaining = {t: int(pl.R_uni[t]) for t in range(TILES)}
                tiles_done = 0
                next_group = 0
                ag_ready = []                           # (group, ready_at_call)
                ag_fired = []                           # (group, fired_at_call)

                def pump_ags(ci, flush=False):
                    # copy AllGather results fired at least one call ago
                    while ag_fired and (flush or ag_fired[0][1] < ci):
                        g_, _ = ag_fired.pop(0)
                        o0, o1_ = grows_out[g_]
                        nc.sync.dma_start(out=table2[o0:o1_, :], in_=cc_ag[g_][:])
                    # fire AllGathers whose cc_in DMA was issued a call ago
                    while ag_ready and (flush or ag_ready[0][1] < ci):
                        g_, _ = ag_ready.pop(0)
                        r0, r1 = grows_in[g_]
                        nc.gpsimd.collective_compute(
                            "AllGather", OP.bypass,
                            replica_groups=[list(range(NC))],
                            ins=[cc_in[r0:r1, :].opt()],
                            outs=[cc_ag[g_][:].opt()],
                        )
                        ag_fired.append((g_, ci))

                for ci, chunks in enumerate(pl.calls):
                    c0, c1 = pl.col_ranges[ci]
                    nch = len(chunks)
                    gbuf = gpool.tile([P, GB, D], bf, tag="g",
                                      name=f"g_{layer}_{ci}")
                    nc.gpsimd.dma_gather(
                        out_ap=gbuf[:, :nch, :],
                        in_ap=table_ap,
                        idxs_ap=idx_t[:, c0:c1],
                        num_idxs=nch * P,
                        num_idxs_reg=nch * P,
                        elem_size=D,
                        single_packet=False,
                        queue_num=ci % NQ,
                    )
                    if layer == 0 and os.environ.get("V2_AG_INTERLEAVE", "1") == "1":
                        pump_ags(ci)
                    for i, ch in enumerate(chunks):
                        t = int(pl.tile_of_chunk[ch])
                        if t not in open_ps:
                            psT = psA.tile([P, D], f32, space="PSUM", tag="agg",
                                           name=f"ps_{layer}_{t}")
                            nc.tensor.matmul(out=psT[:], lhsT=selfstore[:, t, :],
                                             rhs=ident_bf[:], start=True, stop=False)
                            open_ps[t] = psT
                        psT = open_ps[t]
                        remaining[t] -= 1
                        last = remaining[t] == 0
                        nc.tensor.matmul(out=psT[:], lhsT=gbuf[:, i, :],
                                         rhs=ident_bf[:], start=False, stop=last)
                        if last:
                            finish_tile(layer, t, open_ps.pop(t))
                            tiles_done += 1
                            if layer == 0 and next_group < 7 and \
                                    tiles_done == gstart[next_group] + GROUP_SZ[next_group]:
                                r0, r1 = grows_in[next_group]
                                sz = GROUP_SZ[next_group]
                                nc.sync.dma_start(
                                    out=cc_in[r0:r1, :].rearrange(
                                        "(t l) f -> l t f", t=sz),
                                    in_=h1g_store[:, gstart[next_group]:
                                                  gstart[next_group] + sz, :])
                                ag_ready.append((next_group, ci))
                                next_group += 1
                assert not open_ps
                if layer == 0:
                    assert next_group == 7, next_group
                    pump_ags(len(pl.calls), flush=True)
                    pump_ags(len(pl.calls), flush=True)

            run_layer2(0)
            run_layer2(1)

    nc.compile()
    return nc


# ----------------------------------------------------------------------------
# Entry point
# ----------------------------------------------------------------------------

_last_result = None


def kernel(**inputs) -> np.ndarray:
    edge_index = np.asarray(inputs["edge_index"])
    pl = build_plan(edge_index)

    if os.environ.get("KERNEL_EMULATE") == "1":
        return emulate2(pl, inputs)

    from concourse.bass_utils import run_bass_kernel_spmd
    triv = []
    for i in (1, 2):
        triv.append((
            not np.any(np.asarray(inputs[f"b{i}"])),
            np.all(np.asarray(inputs[f"g{i}"]) == 1.0),
            not np.any(np.asarray(inputs[f"beta{i}"])),
        ))
    nc = build_bass(pl, triv)

    in_maps = host_inputs(pl, inputs)

    kw = {}
    if os.environ.get("KERNEL_TRACE") == "1":
        kw = dict(trace=True, trace_cores=[0])
    res = run_bass_kernel_spmd(nc, in_maps, core_ids=list(range(NC)), **kw)
    global _last_result
    _last_result = res

    out = np.zeros((N, D), dtype=np.float32)
    for c in range(NC):
        o = np.asarray(res.results[c]["out"], dtype=np.float32)
        valid = pl.node_at[c] >= 0
        out[pl.node_at[c][valid]] = o[valid]
    return out
